# revision 27
# baseline (speedup 1.0000x reference)
"""ButterflyLinear Trainium2 kernel.

Math insight: every one of the 12 butterfly stages pairs features strictly
within aligned groups of 4 (stage 0 pairs (4k,4k+1),(4k+2,4k+3); stages 1..11
all pair (4k,4k+2),(4k+1,4k+3)).  The whole network therefore collapses
exactly to a block-diagonal linear map with 1024 independent 4x4 blocks:

    out[t, 4k+j] = sum_i x[t, 4k+i] * M_k[i, j] + bias[4k+j]

M is extracted on the host (float64) by pushing the 4 group-basis vectors
through the stage chain.  The device kernel is a feature-major matmul pass:
the host ships x pre-transposed (feature-major tiles, 16KB-contiguous rows),
each 128-feature chunk is one stationary-weight matmul
out_c[of, tok] = W_c.T @ x_c[if, tok] with N=512 tokens moving, bias added
per-partition during the PSUM->SBUF copy, and the host un-transposes the
returned output.  No on-device transposes, no identity, no bias broadcast.

Sharding: data-parallel over tokens, 8192/8 = 1024 tokens per core.
"""

import numpy as np

TOKENS = 8192
N = 4096
DEPTH = 12
NCORES = 8
TOK_PER_CORE = TOKENS // NCORES  # 1024
P = 128                  # partitions
N_CHUNKS = N // P        # 32 feature chunks of 128
GROUP = 4                # chunks per x/out group tile (4*1024 tok = 16KB rows)
N_GROUPS = N_CHUNKS // GROUP   # 8
TBLK = 512               # moving-token block per matmul (fp32 N<=512)
N_TBLK = TOK_PER_CORE // TBLK  # 2


def _apply_stage_np(x, factor, stage):
    B, n = x.shape
    block = 1 << (stage + 1)
    half = block >> 1
    m = n // block
    staged = x.reshape(B, m, half, 2).transpose(0, 1, 3, 2)
    pairs = staged.reshape(B, n // 2, 2)
    t = np.einsum("bnc,ncd->bnd", pairs, factor)
    t = t.reshape(B, m, 2, half).transpose(0, 1, 3, 2)
    return t.reshape(B, n)


def _compose_weights(factors):
    """Return M_cols [4, N] float64: M_cols[i, m] = Mfull[4*(m//4)+i, m]."""
    V = np.zeros((4, N), dtype=np.float64)
    for i in range(4):
        V[i, i::4] = 1.0
    M = V
    f64 = np.asarray(factors, dtype=np.float64)
    for s in range(DEPTH):
        M = _apply_stage_np(M, f64[s], s)
    return M


def _build_wmat(factors):
    """Dense [128, N] fp32 weight: wmat[p, c*128+q] = Mfull[c*128+p, c*128+q].

    Column block c is the (block-diagonal) 128x128 stationary weight for
    feature chunk c (lhsT layout [if, of]); nonzero only where p//4 == q//4.
    """
    M_cols = _compose_weights(factors)  # [4, N]
    wmat = np.zeros((P, N), dtype=np.float64)
    p = np.arange(P)
    q = np.arange(P)
    same_block = (p[:, None] // 4) == (q[None, :] // 4)
    for c in range(N_CHUNKS):
        cols = M_cols[:, c * P:(c + 1) * P]       # [4, 128]
        block = cols[p % 4, :]                    # block[p, q] = M_cols[p%4, q]
        wmat[:, c * P:(c + 1) * P] = np.where(same_block, block, 0.0)
    return np.ascontiguousarray(wmat.astype(np.float32))


_PROG = None


def _get_program():
    global _PROG
    if _PROG is not None:
        return _PROG

    import concourse.mybir as mybir
    import concourse.tile as tile
    from concourse import bacc

    nc = bacc.Bacc("TRN2", target_bir_lowering=False, debug=False,
                   num_devices=NCORES)
    f32 = mybir.dt.float32
    xp_h = nc.dram_tensor("xp", [N_GROUPS, P, GROUP * TOK_PER_CORE], f32,
                          kind="ExternalInput")
    m4_h = nc.dram_tensor("m4", [4, N], f32, kind="ExternalInput")
    sel_h = nc.dram_tensor("sel", [4, P], f32, kind="ExternalInput")
    msk_h = nc.dram_tensor("msk", [P, P], f32, kind="ExternalInput")
    bt_h = nc.dram_tensor("biast", [P, N_CHUNKS], f32, kind="ExternalInput")
    op_h = nc.dram_tensor("outp", [N_GROUPS, P, GROUP * TOK_PER_CORE], f32,
                          kind="ExternalOutput")

    xp = xp_h.ap()
    op = op_h.ap()

    HGRP = GROUP // 2          # 2 chunks per half-group unit
    HCOLS = HGRP * TOK_PER_CORE  # 2048 columns per unit

    with tile.TileContext(nc) as tc:
        with (
            tc.tile_pool(name="singles", bufs=1) as singles,
            tc.tile_pool(name="xin", bufs=4) as xpool,
            tc.tile_pool(name="oout", bufs=4) as opool,
            tc.tile_pool(name="ps", bufs=4, space="PSUM") as pspool,
        ):
            bias_sb = singles.tile([P, N_CHUNKS], f32)
            nc.gpsimd.dma_start(out=bias_sb, in_=bt_h.ap())
            # Stationary weights are built on-device from 130KB of compact
            # data: chunk c = (sel.T @ m4[:, c-slice]) * msk.  The builds
            # are interleaved with the main loop (two chunks per unit) so
            # the PE cycles hide under the DMA stream instead of running
            # cold up front.
            m4_sb = singles.tile([4, N], f32)
            nc.sync.dma_start(out=m4_sb, in_=m4_h.ap())
            sel_sb = singles.tile([4, P], f32)
            nc.sync.dma_start(out=sel_sb, in_=sel_h.ap())
            msk_sb = singles.tile([P, P], f32)
            nc.scalar.dma_start(out=msk_sb, in_=msk_h.ap())
            w_sb = singles.tile([P, N], f32)

            # Units stream loads on nc.sync and stores on nc.scalar, so a
            # store waiting for compute never stalls the next load behind
            # it in the same engine queue.  The last group runs at quarter
            # granularity so the load->store pipeline latency at the tail
            # is halved.  Each unit covers `unit_chunks` feature chunks
            # (1024 tokens per chunk).
            units = [(g * GROUP + h * HGRP, HGRP) for g in range(N_GROUPS - 1)
                     for h in range(2)]
            units += [((N_GROUPS - 1) * GROUP + q, 1) for q in range(GROUP)]

            for ui, (c0, nch) in enumerate(units):
                load_eng = nc.sync if ui % 2 == 0 else nc.scalar
                store_eng = nc.scalar if ui % 2 == 0 else nc.sync
                cols = nch * TOK_PER_CORE
                xg = xpool.tile([P, HCOLS], f32, tag="xg")
                load_eng.dma_start(
                    out=xg[:, 0:cols],
                    in_=xp[c0 // GROUP, :,
                           (c0 % GROUP) * TOK_PER_CORE:
                           (c0 % GROUP) * TOK_PER_CORE + cols])
                for cc in range(nch):
                    c = c0 + cc
                    wp = pspool.tile([P, TBLK], f32, tag="wps")
                    nc.tensor.matmul(wp[:, 0:P], lhsT=sel_sb,
                                     rhs=m4_sb[:, c * P:(c + 1) * P],
                                     start=True, stop=True)
                    nc.vector.tensor_mul(
                        w_sb[:, c * P:(c + 1) * P], wp[:, 0:P], msk_sb)
                og = opool.tile([P, HCOLS], f32, tag="og")
                for cc in range(nch):
                    c = c0 + cc
                    for tb in range(N_TBLK):
                        ps = pspool.tile([P, TBLK], f32)
                        nc.tensor.matmul(
                            ps,
                            lhsT=w_sb[:, c * P:(c + 1) * P],
                            rhs=xg[:, cc * TOK_PER_CORE + tb * TBLK:
                                   cc * TOK_PER_CORE + (tb + 1) * TBLK],
                            start=True, stop=True,
                        )
                        dst = og[:, cc * TOK_PER_CORE + tb * TBLK:
                                 cc * TOK_PER_CORE + (tb + 1) * TBLK]
                        bcol = bias_sb[:, c:c + 1]
                        # All PSUM->SBUF copies on DVE: the ACT sequencer
                        # is the store-DMA issuer, keep it free.
                        nc.vector.tensor_scalar_add(dst, ps, bcol)
                store_eng.dma_start(
                    out=op[c0 // GROUP, :,
                           (c0 % GROUP) * TOK_PER_CORE:
                           (c0 % GROUP) * TOK_PER_CORE + cols],
                    in_=og[:, 0:cols])

    nc.compile()
    _PROG = nc
    return nc


def _prep_core_input(xs):
    """[1024, 4096] token-major -> [8, 128, 4096] feature-major group tiles.

    xprep[g, p, cc*1024 + t] = xs[t, (4g+cc)*128 + p]
    """
    xt = xs.T.reshape(N_GROUPS, GROUP, P, TOK_PER_CORE)   # [g][cc][p][t]
    return np.ascontiguousarray(
        xt.transpose(0, 2, 1, 3).reshape(N_GROUPS, P, GROUP * TOK_PER_CORE))


def _unprep_core_output(outp):
    """Inverse of _prep_core_input for the output tensor."""
    o = outp.reshape(N_GROUPS, P, GROUP, TOK_PER_CORE).transpose(0, 2, 1, 3)
    return o.reshape(N, TOK_PER_CORE).T   # [1024, 4096] token-major view


def kernel(x, factors, bias):
    from concourse.bass_utils import run_bass_kernel_spmd

    x = np.asarray(x, dtype=np.float32)
    factors = np.asarray(factors, dtype=np.float32)
    bias_np = np.asarray(bias, dtype=np.float32)
    assert x.shape == (TOKENS, N)

    m4 = np.ascontiguousarray(_compose_weights(factors).astype(np.float32))
    pidx = np.arange(P)
    sel = np.ascontiguousarray(
        (pidx[None, :] % 4 == np.arange(4)[:, None]).astype(np.float32))
    msk = np.ascontiguousarray(
        ((pidx[:, None] // 4) == (pidx[None, :] // 4)).astype(np.float32))
    biast = np.ascontiguousarray(bias_np.reshape(N_CHUNKS, P).T)

    nc = _get_program()
    in_maps = []
    for c in range(NCORES):
        in_maps.append({
            "xp": _prep_core_input(x[c * TOK_PER_CORE:(c + 1) * TOK_PER_CORE]),
            "m4": m4,
            "sel": sel,
            "msk": msk,
            "biast": biast,
        })
    res = run_bass_kernel_spmd(nc, in_maps, core_ids=list(range(NCORES)))
    out = np.empty((TOKENS, N), dtype=np.float32)
    for c in range(NCORES):
        out[c * TOK_PER_CORE:(c + 1) * TOK_PER_CORE] = _unprep_core_output(
            res.results[c]["outp"])
    return out


# revision 33
# speedup vs baseline: 1.0289x; 1.0289x over previous
"""ButterflyLinear Trainium2 kernel.

Math insight: every one of the 12 butterfly stages pairs features strictly
within aligned groups of 4 (stage 0 pairs (4k,4k+1),(4k+2,4k+3); stages 1..11
all pair (4k,4k+2),(4k+1,4k+3)).  The whole network therefore collapses
exactly to a block-diagonal linear map with 1024 independent 4x4 blocks:

    out[t, 4k+j] = sum_i x[t, 4k+i] * M_k[i, j] + bias[4k+j]

M is extracted on the host (float64) by pushing the 4 group-basis vectors
through the stage chain.  The device kernel is a feature-major matmul pass:
the host ships x pre-transposed (feature-major tiles, 16KB-contiguous rows),
each 128-feature chunk is one stationary-weight matmul
out_c[of, tok] = W_c.T @ x_c[if, tok] with N=512 tokens moving, bias added
per-partition during the PSUM->SBUF copy, and the host un-transposes the
returned output.  No on-device transposes, no identity, no bias broadcast.

Sharding: data-parallel over tokens, 8192/8 = 1024 tokens per core.
"""

import numpy as np

TOKENS = 8192
N = 4096
DEPTH = 12
NCORES = 8
TOK_PER_CORE = TOKENS // NCORES  # 1024
P = 128                  # partitions
N_CHUNKS = N // P        # 32 feature chunks of 128
GROUP = 4                # chunks per x/out group tile (4*1024 tok = 16KB rows)
N_GROUPS = N_CHUNKS // GROUP   # 8
TBLK = 512               # moving-token block per matmul (fp32 N<=512)
N_TBLK = TOK_PER_CORE // TBLK  # 2


def _apply_stage_np(x, factor, stage):
    B, n = x.shape
    block = 1 << (stage + 1)
    half = block >> 1
    m = n // block
    staged = x.reshape(B, m, half, 2).transpose(0, 1, 3, 2)
    pairs = staged.reshape(B, n // 2, 2)
    t = np.einsum("bnc,ncd->bnd", pairs, factor)
    t = t.reshape(B, m, 2, half).transpose(0, 1, 3, 2)
    return t.reshape(B, n)


def _compose_weights(factors):
    """Return M_cols [4, N] float64: M_cols[i, m] = Mfull[4*(m//4)+i, m]."""
    V = np.zeros((4, N), dtype=np.float64)
    for i in range(4):
        V[i, i::4] = 1.0
    M = V
    f64 = np.asarray(factors, dtype=np.float64)
    for s in range(DEPTH):
        M = _apply_stage_np(M, f64[s], s)
    return M


_PROG = None


def _get_program():
    global _PROG
    if _PROG is not None:
        return _PROG

    import concourse.mybir as mybir
    import concourse.tile as tile
    from concourse import bacc

    nc = bacc.Bacc("TRN2", target_bir_lowering=False, debug=False,
                   num_devices=NCORES)
    f32 = mybir.dt.float32
    xp_h = nc.dram_tensor("xp", [N_GROUPS, P, GROUP * TOK_PER_CORE], f32,
                          kind="ExternalInput")
    m4_h = nc.dram_tensor("m4", [4, N], f32, kind="ExternalInput")
    sel_h = nc.dram_tensor("sel", [4, P], f32, kind="ExternalInput")
    msk_h = nc.dram_tensor("msk", [P, P], f32, kind="ExternalInput")
    bt_h = nc.dram_tensor("biast", [P, N_CHUNKS], f32, kind="ExternalInput")
    op_h = nc.dram_tensor("outp", [N_GROUPS, P, GROUP * TOK_PER_CORE], f32,
                          kind="ExternalOutput")

    xp = xp_h.ap()
    op = op_h.ap()

    HGRP = GROUP // 2          # 2 chunks per half-group unit
    HCOLS = HGRP * TOK_PER_CORE  # 2048 columns per unit

    with tile.TileContext(nc) as tc:
        with (
            tc.tile_pool(name="singles", bufs=1) as singles,
            tc.tile_pool(name="xin", bufs=4) as xpool,
            tc.tile_pool(name="oout", bufs=4) as opool,
            tc.tile_pool(name="ps", bufs=6, space="PSUM") as pspool,
            tc.tile_pool(name="wps", bufs=2, space="PSUM") as wpspool,
        ):
            bias_sb = singles.tile([P, N_CHUNKS], f32)
            nc.gpsimd.dma_start(out=bias_sb, in_=bt_h.ap())
            # Stationary weights are built on-device from 130KB of compact
            # data: chunk c = (sel.T @ m4[:, c-slice]) * msk.  The builds
            # are interleaved with the main loop (two chunks per unit) so
            # the PE cycles hide under the DMA stream instead of running
            # cold up front.
            m4_sb = singles.tile([4, N], f32)
            nc.sync.dma_start(out=m4_sb, in_=m4_h.ap())
            sel_sb = singles.tile([4, P], f32)
            nc.sync.dma_start(out=sel_sb, in_=sel_h.ap())
            msk_sb = singles.tile([P, P], f32)
            nc.scalar.dma_start(out=msk_sb, in_=msk_h.ap())
            w_sb = singles.tile([P, N], f32)

            # Units stream loads on nc.sync and stores on nc.scalar, so a
            # store waiting for compute never stalls the next load behind
            # it in the same engine queue.  The last group runs at quarter
            # granularity so the load->store pipeline latency at the tail
            # is halved.  Each unit covers `unit_chunks` feature chunks
            # (1024 tokens per chunk).
            units = [(g * GROUP + h * HGRP, HGRP) for g in range(N_GROUPS - 1)
                     for h in range(2)]
            units += [((N_GROUPS - 1) * GROUP + q, 1) for q in range(GROUP)]

            for c0, nch in units:
                cols = nch * TOK_PER_CORE
                xg = xpool.tile([P, HCOLS], f32, tag="xg")
                nc.sync.dma_start(
                    out=xg[:, 0:cols],
                    in_=xp[c0 // GROUP, :,
                           (c0 % GROUP) * TOK_PER_CORE:
                           (c0 % GROUP) * TOK_PER_CORE + cols])
                for cc in range(nch):
                    c = c0 + cc
                    wp = wpspool.tile([P, P], f32)
                    nc.tensor.matmul(wp, lhsT=sel_sb,
                                     rhs=m4_sb[:, c * P:(c + 1) * P],
                                     start=True, stop=True)
                    nc.vector.tensor_mul(
                        w_sb[:, c * P:(c + 1) * P], wp, msk_sb)
                og = opool.tile([P, HCOLS], f32, tag="og")
                for cc in range(nch):
                    c = c0 + cc
                    for tb in range(N_TBLK):
                        ps = pspool.tile([P, TBLK], f32)
                        nc.tensor.matmul(
                            ps,
                            lhsT=w_sb[:, c * P:(c + 1) * P],
                            rhs=xg[:, cc * TOK_PER_CORE + tb * TBLK:
                                   cc * TOK_PER_CORE + (tb + 1) * TBLK],
                            start=True, stop=True,
                        )
                        dst = og[:, cc * TOK_PER_CORE + tb * TBLK:
                                 cc * TOK_PER_CORE + (tb + 1) * TBLK]
                        bcol = bias_sb[:, c:c + 1]
                        # All PSUM->SBUF copies on DVE: the ACT sequencer
                        # is the store-DMA issuer, keep it free.
                        nc.vector.tensor_scalar_add(dst, ps, bcol)
                nc.scalar.dma_start(
                    out=op[c0 // GROUP, :,
                           (c0 % GROUP) * TOK_PER_CORE:
                           (c0 % GROUP) * TOK_PER_CORE + cols],
                    in_=og[:, 0:cols])

    nc.compile()
    _PROG = nc
    return nc


def _prep_core_input(xs):
    """[1024, 4096] token-major -> [8, 128, 4096] feature-major group tiles.

    xprep[g, p, cc*1024 + t] = xs[t, (4g+cc)*128 + p]
    """
    xt = xs.T.reshape(N_GROUPS, GROUP, P, TOK_PER_CORE)   # [g][cc][p][t]
    return np.ascontiguousarray(
        xt.transpose(0, 2, 1, 3).reshape(N_GROUPS, P, GROUP * TOK_PER_CORE))


def _unprep_core_output(outp):
    """Inverse of _prep_core_input for the output tensor."""
    o = outp.reshape(N_GROUPS, P, GROUP, TOK_PER_CORE).transpose(0, 2, 1, 3)
    return o.reshape(N, TOK_PER_CORE).T   # [1024, 4096] token-major view


def kernel(x, factors, bias):
    from concourse.bass_utils import run_bass_kernel_spmd

    x = np.asarray(x, dtype=np.float32)
    factors = np.asarray(factors, dtype=np.float32)
    bias_np = np.asarray(bias, dtype=np.float32)
    assert x.shape == (TOKENS, N)

    m4 = np.ascontiguousarray(_compose_weights(factors).astype(np.float32))
    pidx = np.arange(P)
    sel = np.ascontiguousarray(
        (pidx[None, :] % 4 == np.arange(4)[:, None]).astype(np.float32))
    msk = np.ascontiguousarray(
        ((pidx[:, None] // 4) == (pidx[None, :] // 4)).astype(np.float32))
    biast = np.ascontiguousarray(bias_np.reshape(N_CHUNKS, P).T)

    nc = _get_program()
    in_maps = []
    for c in range(NCORES):
        in_maps.append({
            "xp": _prep_core_input(x[c * TOK_PER_CORE:(c + 1) * TOK_PER_CORE]),
            "m4": m4,
            "sel": sel,
            "msk": msk,
            "biast": biast,
        })
    res = run_bass_kernel_spmd(nc, in_maps, core_ids=list(range(NCORES)))
    out = np.empty((TOKENS, N), dtype=np.float32)
    for c in range(NCORES):
        out[c * TOK_PER_CORE:(c + 1) * TOK_PER_CORE] = _unprep_core_output(
            res.results[c]["outp"])
    return out
